# revision 16
# baseline (speedup 1.0000x reference)
"""Trainium2 Bass kernel for nn_AttentionModule (SAGAN-style 2D self-attention).

Per-sample computation (B=8 samples, one per NeuronCore, data-parallel):
    q = Wq @ x + bq         (32, 4096)
    k = Wk @ x + bk         (32, 4096)
    v = Wv @ x + bv         (256, 4096)
    attn = softmax(q^T k)   (4096, 4096), softmax over last dim
    y = v @ attn^T + x      (256, 4096)

Strategy per core:
  - Scores computed TRANSPOSED: Lt[m, n] = sum_d k[d,m] q[d,n], so the
    exp'd scores P land in SBUF with the contraction dim (m) on partitions,
    directly usable as the stationary operand of the AV matmul.
  - No softmax max-subtraction (|logits| < ~25, exp fits fp32 comfortably).
  - Softmax denominator comes free: the AV streaming operand v' carries an
    extra ones-column, so outT[n, 256] = colsum(P).
  - Logits matmuls (K = D = 32) are 2x row-packed via tile_position, fp16.
  - AV in bf16 (P needs range up to ~5e10: bf16, not fp16); projections,
    logits, transposes in fp16 (f32r matmuls measured 2-4x slow on HW due
    to serialized self-loading weight loads).
  - Softmax-exp on ACT, [128,1024] per 2-bank PSUM logits group, double
    buffered so the next pack's LDWEIGHTS never waits on the current exp.
  - Final: per n-block normalize by 1/colsum, PE-transpose back to [c, n],
    add fp32 residual x, DMA out.

Host-side prep: weights are passed pre-transposed/replicated per the SBUF
layouts the kernel wants (kernel() receives full unsharded inputs).
"""

import numpy as np

import concourse.bacc as bacc
import concourse.bass as bass
import concourse.mybir as mybir
import concourse.tile as tile
from concourse.masks import make_identity

B, C, D = 8, 256, 32
HW = 4096                      # 64*64 pixels
NCH = 8                        # n-chunks of 512
CHUNK = 512
NB = 128                       # n-block
MB = 128                       # m-block
NMB = HW // MB                 # 32 m-blocks
VW = 258                       # v' row width: 256 c + colsum + pad
GS = [3] * 10 + [2]            # m-blocks per logits group (sum = 32)
GOFF = [0]
for _g in GS:
    GOFF.append(GOFF[-1] + _g)  # group -> first m-block
F32 = mybir.dt.float32
BF16 = mybir.dt.bfloat16
FP16 = mybir.dt.float16
AF = mybir.ActivationFunctionType


def build_nc():
    nc = bacc.Bacc("TRN2", target_bir_lowering=False, debug=False)
    t = {}
    t["x"] = nc.dram_tensor("x", [C, HW], F32, kind="ExternalInput").ap()
    t["wq3"] = nc.dram_tensor("wq3", [C, 96], FP16, kind="ExternalInput").ap()
    t["wk3"] = nc.dram_tensor("wk3", [C, 96], FP16, kind="ExternalInput").ap()
    t["bq3"] = nc.dram_tensor("bq3", [96, 1], F32, kind="ExternalInput").ap()
    t["bk3"] = nc.dram_tensor("bk3", [96, 1], F32, kind="ExternalInput").ap()
    t["wvtp"] = nc.dram_tensor("wvtp", [C, VW], FP16, kind="ExternalInput").ap()
    t["bvp"] = nc.dram_tensor("bvp", [1, VW], FP16, kind="ExternalInput").ap()
    t["ones1"] = nc.dram_tensor("ones1", [1, 128], FP16, kind="ExternalInput").ap()
    t["y"] = nc.dram_tensor("y", [C, HW], F32, kind="ExternalOutput").ap()

    with tile.TileContext(nc) as tc:
        _emit(nc, tc, t)
    nc.compile()
    return nc


def _emit(nc, tc, t):
    with (
        tc.tile_pool(name="const", bufs=1) as const,
        tc.tile_pool(name="sb", bufs=1) as sb,
        tc.tile_pool(name="stage", bufs=2) as stage,
    ):
        # ---- constants / weights -------------------------------------
        ident = const.tile([128, 128], FP16)
        make_identity(nc, ident)
        ones1 = const.tile([1, 128], FP16)
        nc.sync.dma_start(ones1, t["ones1"])
        wq3 = const.tile([128, 2, 96], FP16)   # [c', cc, 3x32 q-weights]
        wk3 = const.tile([128, 2, 96], FP16)
        wvtp = const.tile([128, 2, VW], FP16)  # [c', cc, 256 v-w + 0-cols]
        bq3 = const.tile([96, 1], F32)
        bk3 = const.tile([96, 1], F32)
        bvp = const.tile([1, VW], FP16)
        for cc in range(2):
            nc.sync.dma_start(wq3[:, cc, :], t["wq3"][128 * cc:128 * (cc + 1), :])
            nc.sync.dma_start(wk3[:, cc, :], t["wk3"][128 * cc:128 * (cc + 1), :])
            nc.sync.dma_start(wvtp[:, cc, :], t["wvtp"][128 * cc:128 * (cc + 1), :])
        nc.sync.dma_start(bq3, t["bq3"])
        nc.sync.dma_start(bk3, t["bk3"])
        nc.sync.dma_start(bvp, t["bvp"])

        # ---- persistent SBUF tensors ---------------------------------
        x0 = sb.tile([128, HW], F32)           # x rows 0:128 (residual)
        x1 = sb.tile([128, HW], F32)           # x rows 128:256
        x16 = sb.tile([128, 2, HW], FP16)      # fp16 copy for projections
        q3 = sb.tile([96, HW], FP16)           # q replicated 3x on partitions
        k3 = sb.tile([96, HW], FP16)
        vp = sb.tile([128, NMB * VW], BF16)    # v' tiles: [m-chunk 128, VW]
        pbuf = [sb.tile([128, 16 * 1024], BF16, tag=f"p{i}", name=f"p{i}")
                for i in range(2)]

        xc = [x0, x1]
        for ch in range(NCH):
            s = slice(CHUNK * ch, CHUNK * (ch + 1))
            nc.sync.dma_start(x0[:, s], t["x"][0:128, s])
            nc.gpsimd.dma_start(x1[:, s], t["x"][128:256, s])
            # fp16 casts split between ACT and DVE
            nc.scalar.activation(x16[:, 0, s], x0[:, s], AF.Identity)
            nc.vector.tensor_copy(x16[:, 1, s], x1[:, s])

        # ---- phase 0: k projection + q chunk 0 (fp16) ----------------
        def qk_proj(pool, w3, b3, dst, ch, tag, bufs=None):
            s = slice(CHUNK * ch, CHUNK * (ch + 1))
            pt = pool.tile([96, CHUNK], F32, tag=tag, name="pt", bufs=bufs)
            for cc in range(2):
                nc.tensor.matmul(
                    pt, w3[:, cc, :], x16[:, cc, s],
                    start=(cc == 0), stop=(cc == 1),
                )
            nc.vector.tensor_scalar_add(dst[:, s], pt, b3)

        with tc.tile_pool(name="ps0", bufs=2, space="PSUM") as ps0:
            qk_proj(ps0, wk3, bk3, k3, 0, "proj")
            qk_proj(ps0, wq3, bq3, q3, 0, "proj")

        # ---- main loop -----------------------------------------------
        # PSUM: lt 3-bank x2 bufs = 6 banks; "avtr" shared tag (av accum /
        # transpose out / v'-proj) 1 bank x2 bufs = 2 banks. Total 8.
        with tc.tile_pool(name="ps1", bufs=1, space="PSUM") as ps1:

            def pgoff(mc):
                g = min(mc // 3, 10)
                return g, 1536 * g + CHUNK * (mc - 3 * g)

            def logits_group(ch, g):
                """GS[g] row-packed matmuls (m-blocks GOFF[g]..) + exp."""
                sz = GS[g]
                lt = ps1.tile([128, 1536], F32, tag="lt", bufs=2, name="lt")
                ns = slice(CHUNK * ch, CHUNK * (ch + 1))
                for r in range(sz):
                    mb = GOFF[g] + r
                    nc.tensor.matmul(
                        lt[:, CHUNK * r:CHUNK * (r + 1)],
                        k3[32 * r:32 * (r + 1), MB * mb:MB * (mb + 1)],
                        q3[32 * r:32 * (r + 1), ns],
                        start=True, stop=True, tile_position=(32 * r, 0),
                    )
                dst = pbuf[ch % 2][:, 1536 * g:1536 * g + CHUNK * sz]
                nc.scalar.activation(dst, lt[:, 0:CHUNK * sz], AF.Exp)

            def vt_unit(mb):
                """v' tile mb: 3 matmuls + copy (chunk-0 filler work)."""
                ms = slice(MB * mb, MB * (mb + 1))
                vt = ps1.tile([128, VW], F32, tag="avtr", bufs=2, name="vt")
                for cc in range(2):
                    nc.tensor.matmul(
                        vt, x16[:, cc, ms], wvtp[:, cc, :],
                        start=(cc == 0), stop=False,
                    )
                nc.tensor.matmul(vt, ones1, bvp, start=False, stop=True)
                if mb % 4 == 0:
                    nc.scalar.activation(vp[:, VW * mb:VW * (mb + 1)], vt,
                                         AF.Identity)
                else:
                    nc.vector.tensor_copy(vp[:, VW * mb:VW * (mb + 1)], vt)

            def av_unit(ch, j, mc):
                g, off = pgoff(mc)
                nc.tensor.matmul(
                    t["avps"], pbuf[ch % 2][:, off + NB * j:off + NB * (j + 1)],
                    vp[:, VW * mc:VW * (mc + 1)],
                    start=(mc == 0), stop=(mc == 31),
                )

            def finalize_p1(ch, j):
                avps = t["avps"]
                recip = stage.tile([128, 1], F32, tag="recip", name="recip")
                nc.vector.reciprocal(recip, avps[:, 256:257])
                normt = stage.tile([128, 256], FP16, tag="normt", name="normt")
                nc.vector.tensor_scalar_mul(normt, avps[:, 0:256], recip)
                return normt

            def finalize_p2(ch, j, ysb, normt):
                trp = ps1.tile([128, 256], FP16, tag="avtr", bufs=2, name="trp")
                for cb in range(2):
                    nc.tensor.transpose(
                        trp[:, 128 * cb:128 * (cb + 1)],
                        normt[:, 128 * cb:128 * (cb + 1)], ident)
                nb = 4 * ch + j
                for cb in range(2):
                    nc.vector.tensor_tensor(
                        out=ysb[cb][:, NB * j:NB * (j + 1)],
                        in0=trp[:, 128 * cb:128 * (cb + 1)],
                        in1=xc[cb][:, NB * nb:NB * (nb + 1)],
                        op=mybir.AluOpType.add,
                    )

            NG = len(GS)  # 11 logits groups per chunk
            # k3 chunks required before logits pack g can run (cols 384g..)
            KREQ = [min((128 * GOFF[g + 1] - 1) // CHUNK, NCH - 1)
                    for g in range(NG)]

            for ch in range(NCH + 1):
                # filler units for this pipeline stage:
                #  ch == 0  -> deferred k3(1..7), q3(1..7), 32 v'-proj units
                #  ch >= 1  -> 128 AV matmuls of chunk ch-1 (+finalize/4)
                if ch == 0:
                    fillers = ([("k3", c) for c in range(1, NCH)]
                               + [("q3", c) for c in range(1, NCH)]
                               + [("vt", mb) for mb in range(NMB)])
                    n_units = len(fillers)
                else:
                    n_units = 128
                ysb = None
                if ch > 0:
                    ysb = [stage.tile([128, CHUNK], F32, tag=f"y{cb}",
                                      name=f"ysb{cb}") for cb in range(2)]
                g_next = 0
                pending = None  # (j, normt) awaiting transposes+adds
                for u in range(n_units):
                    if ch < NCH:
                        while g_next < NG and (
                                g_next <= (u * NG) // n_units
                                or (ch == 0 and g_next == 0)):
                            if ch == 0 and any(
                                    f[0] == "k3" and f[1] <= KREQ[g_next]
                                    for f in fillers[u:]):
                                break  # k3 dependency not yet emitted
                            logits_group(ch, g_next)
                            g_next += 1
                    if ch == 0:
                        kind, a = fillers[u]
                        if kind == "k3":
                            qk_proj(ps1, wk3, bk3, k3, a, "avtr", bufs=2)
                        elif kind == "q3":
                            qk_proj(ps1, wq3, bq3, q3, a, "avtr", bufs=2)
                        else:
                            vt_unit(a)
                    else:
                        j, mc = divmod(u, 32)
                        if j == 0 and mc == 0:
                            t["avps"] = ps1.tile([128, VW], F32, tag="avtr",
                                                 bufs=2, name="avps")
                        av_unit(ch - 1, j, mc)
                        if mc == 6 and pending is not None:
                            finalize_p2(ch - 1, pending[0], ysb, pending[1])
                            pending = None
                        if mc == 31:
                            normt = finalize_p1(ch - 1, j)
                            if j < 3:
                                t["avps"] = ps1.tile([128, VW], F32,
                                                     tag="avtr", bufs=2,
                                                     name="avps")
                                pending = (j, normt)
                            else:
                                finalize_p2(ch - 1, j, ysb, normt)
                if ch < NCH:
                    while g_next < NG:
                        logits_group(ch, g_next)
                        g_next += 1
                if ch > 0:
                    s = slice(CHUNK * (ch - 1), CHUNK * ch)
                    nc.sync.dma_start(t["y"][0:128, s], ysb[0])
                    nc.sync.dma_start(t["y"][128:256, s], ysb[1])


# ---------------------------------------------------------------------
# host-side wrapper
# ---------------------------------------------------------------------
_CACHE = {}


def _prep_shared(Wq, bq, Wk, bk, Wv, bv):
    wq3 = np.tile(np.ascontiguousarray(Wq.T), (1, 3)).astype(np.float16)
    wk3 = np.tile(np.ascontiguousarray(Wk.T), (1, 3)).astype(np.float16)
    bq3 = np.tile(bq, 3).reshape(96, 1).astype(np.float32)
    bk3 = np.tile(bk, 3).reshape(96, 1).astype(np.float32)
    wvtp = np.concatenate(
        [Wv.T, np.zeros((C, 2), np.float32)], axis=1).astype(np.float16)
    bvp = np.concatenate([bv, [1.0, 0.0]]).reshape(1, VW).astype(np.float16)
    return {"wq3": np.ascontiguousarray(wq3), "wk3": np.ascontiguousarray(wk3),
            "bq3": bq3, "bk3": bk3,
            "wvtp": np.ascontiguousarray(wvtp), "bvp": bvp,
            "ones1": np.ones((1, 128), np.float16)}


def make_in_maps(x, Wq, bq, Wk, bk, Wv, bv):
    x = np.asarray(x, dtype=np.float32).reshape(B, C, HW)
    shared = _prep_shared(*(np.asarray(a, dtype=np.float32)
                            for a in (Wq, bq, Wk, bk, Wv, bv)))
    return [{"x": np.ascontiguousarray(x[b]), **shared} for b in range(B)]


def kernel(x, Wq, bq, Wk, bk, Wv, bv):
    from concourse.bass_utils import run_bass_kernel_spmd

    in_maps = make_in_maps(x, Wq, bq, Wk, bk, Wv, bv)
    if "nc" not in _CACHE:
        _CACHE["nc"] = build_nc()
    res = run_bass_kernel_spmd(_CACHE["nc"], in_maps, core_ids=list(range(B)))
    y = np.stack([res.results[b]["y"] for b in range(B)])
    return y.reshape(B, C, 64, 64).astype(np.float32)
